# revision 4
# baseline (speedup 1.0000x reference)
"""RNN-T JointNet kernel for 8 Trainium2 NeuronCores.

Math: out[b,t,u,:] = gelu_tanh(concat(enc[b,t], dec[b,u])) @ W_fc^T + b_fc
Since gelu is elementwise, gelu(concat(a,b)) = concat(gelu(a), gelu(b)), so
  out[b,t,u,:] = P_enc[b,t,:] + P_dec[b,u,:]
with P_enc = gelu(enc) @ W_fc[:, :512]^T + b_fc  (small matmul)
     P_dec = gelu(dec) @ W_fc[:, 512:]^T         (small matmul)
The dominant cost is streaming the (B,T,U,V) output to HBM. The output is
stored as bf16 (compute is already bf16; rel err ~2e-3) and upcast to f32 on
the host, halving HBM store traffic vs f32 (~20MB/core, ~56us at 358GB/s).

Sharding: 8 cores = 2 batch-pairs x 4 u-quarters. Core c -> bp = c//4
(batches {0,1} or {2,3}), uq = c%4 with u range [25*uq, 25*uq+26) (26 rows,
1-row overlap between quarters; quarter q>0 contributes local rows 1..25).
Per-core row space: 600 (b,t) rows laid out as 5 chunks of 120 partitions
with the two batches INTERLEAVED across partitions: chunk c, partition p ->
batch 2*bp + p%2, t = c*60 + p//2. This avoids any ragged 44-row t-tail
(no gpsimd elementwise, which is the baseline bottleneck at 2.6 cyc/elem),
keeps DVE lane utilization at 94%, and keeps SDMA engine load balanced
(max engine carries 8/120 of bytes < its 1/16 share of 358GB/s).

Per-u pipeline: PE builds the per-partition addend P_dec[batch(p), u] into a
[120,640] f32 PSUM tile via a K=2 matmul (even/odd partition masks x the two
batches' P_dec rows, both batches' rows living in a [2, 26*V] tile); ACT
copies it to a bf16 bct tile; one DVE tensor_tensor (bf16 2x mode, ~1.7us)
adds pe[120,5,640] (P_enc+bias, bf16) + bct (stride-0 broadcast over the 5
chunks) into the out tile j-slice. Stores: one 1.54MB fully contiguous DMA
per 2-u block, alternating HWDGE rings. DVE ~45us and ACT ~18us hide under
the ~56us of stores.
"""

import numpy as np

B, T, U = 4, 300, 101
D = 512
V = 640
UCORE = 26  # u rows per core (4 quarters x 25 + 1 overlap row)
NCORES = 8
UB = 2  # u rows per store block (26 = 13 * 2)
NBLK = UCORE // UB
NCH = 5  # row chunks of 120 partitions (600 = 2 batches x 300 t)
PCH = 120
NWARM = 45  # dummy PE transposes to hold the HAM clock-gate open

LAST_RESULT = None  # BassKernelResults of the most recent run (for test.py)
RUN_KWARGS = {}  # extra kwargs test.py may inject (e.g. tmpdir for traces)

_cache = {}


def _build():
    import concourse.mybir as mybir
    from concourse import bacc, masks
    from concourse.tile import TileContext

    f32 = mybir.dt.float32
    bf16 = mybir.dt.bfloat16
    AF = mybir.ActivationFunctionType

    nc = bacc.Bacc()
    # host pre-tiled: enc[p, c, :] = enc row (batch p%2, t = c*60 + p//2)
    enc_d = nc.dram_tensor("enc", [PCH, NCH, D], bf16, kind="ExternalInput")
    # dec[s*26 + u, :] = dec row (batch s, u0 + u)
    dec_d = nc.dram_tensor("dec", [2 * UCORE, D], bf16, kind="ExternalInput")
    # host pre-tiled: wT[p, c, :] = W_fc.T row d = c*128+p (c 0-3 enc, 4-7 dec)
    wT_d = nc.dram_tensor("wT", [128, 8, V], bf16, kind="ExternalInput")
    bias_d = nc.dram_tensor("bias", [1, V], bf16, kind="ExternalInput")
    # masks2[0, p] = p%2==0, masks2[1, p] = p%2==1 (batch-select for K=2 bcast)
    masks_d = nc.dram_tensor("masks2", [2, 128], bf16, kind="ExternalInput")
    # output laid out exactly like the SBUF tiles so every store is one
    # fully contiguous DRAM write; host un-permutes + upcasts to f32.
    # outD[bi, p, j, c, v] = out[batch p%2, t = c*60 + p//2, u = UB*bi + j, v]
    outD = nc.dram_tensor("outD", [NBLK, PCH, UB, NCH, V], bf16, kind="ExternalOutput")

    vchunks = [(0, 512), (512, V - 512)]

    with TileContext(nc) as tc:
        with (
            tc.tile_pool(name="const", bufs=1) as constp,
            tc.tile_pool(name="work", bufs=2) as work,
            tc.tile_pool(name="persist", bufs=1) as persist,
            tc.tile_pool(name="outp", bufs=3) as outp,
            tc.tile_pool(name="bctp", bufs=3) as bctp,
            tc.tile_pool(name="psum", bufs=1, space="PSUM") as psum,
        ):
            # input loads: small gelu inputs first (they head the compute
            # chains), split across both HWDGE rings; w on the SWDGE ring so
            # it dispatches early and overlaps the HWDGE input loads
            dt_in = work.tile([128, D], bf16, tag="ld", name="dt_in")
            nc.sync.dma_start(dt_in[: 2 * UCORE, :], dec_d[:, :])
            et = work.tile([PCH, NCH, D], bf16, tag="lde", name="et")
            nc.scalar.dma_start(et[:, :, :], enc_d[:, :, :])
            w_bf = persist.tile([128, 8, V], bf16, tag="w")
            nc.gpsimd.dma_start(w_bf[:, :, :], wT_d[:, :, :])
            bias_sb = constp.tile([1, V], bf16)
            nc.scalar.dma_start(bias_sb[:], bias_d[:])
            masks2 = constp.tile([2, 128], bf16)
            nc.scalar.dma_start(masks2[:], masks_d[:])

            ident = constp.tile([128, 128], bf16)
            masks.make_identity(nc, ident[:])
            ones1 = constp.tile([1, 128], bf16)
            nc.gpsimd.memset(ones1[:], 1.0)

            # dummy PE ops: absorb the gpsimd-sem wait AND keep the PE HAM
            # activity window busy until real matmuls arrive (2.4GHz vs 1.2)
            warm = psum.tile([128, 128], bf16, tag="tr", bufs=2)
            for _ in range(NWARM):
                nc.tensor.transpose(warm[:, :], ident[:, :], ident[:, :])

            # gelu: dec first (heads the deeper P_dec->rows2 chain)
            gdec = persist.tile([128, D], bf16, tag="gdec")
            nc.scalar.activation(gdec[: 2 * UCORE, :], dt_in[: 2 * UCORE, :], AF.Gelu_apprx_tanh)
            genc = persist.tile([PCH, NCH, D], bf16, tag="genc")
            nc.scalar.activation(genc[:, :, :], et[:, :, :], AF.Gelu_apprx_tanh)

            # transpose to [d, row]; psum->SBUF copies on the DVE
            gdecT = [persist.tile([128, 2 * UCORE], bf16, tag=f"gdecT{d}", name=f"gdecT{d}") for d in range(4)]
            gencT = [persist.tile([128, NCH * PCH], bf16, tag=f"gencT{d}", name=f"gencT{d}") for d in range(4)]
            for dch in range(4):
                dsl = slice(dch * 128, (dch + 1) * 128)
                ps = psum.tile([128, 128], bf16, tag="tr", bufs=2)
                nc.tensor.transpose(ps[:, : 2 * UCORE], gdec[: 2 * UCORE, dsl], ident[: 2 * UCORE, : 2 * UCORE])
                nc.vector.tensor_copy(gdecT[dch][:, : 2 * UCORE], ps[:, : 2 * UCORE])
            for dch in range(4):
                dsl = slice(dch * 128, (dch + 1) * 128)
                for i in range(NCH):
                    ps = psum.tile([128, 128], bf16, tag="tr", bufs=2)
                    nc.tensor.transpose(ps[:, :PCH], genc[:PCH, i, dsl], ident[:PCH, :PCH])
                    nc.vector.tensor_copy(gencT[dch][:, i * PCH : (i + 1) * PCH], ps[:, :PCH])

            # P_dec [52,640] bf16 -> SBUF->SBUF DMA relayout to a [2, 26*V]
            # row tile: rows2[s, u*V + v] = P_dec[batch s, u0+u, v]
            pd_bf = persist.tile([2 * UCORE, V], bf16, tag="pd")
            ps = psum.tile([128, V], f32, tag="bc", bufs=3)
            for v0, vn in vchunks:
                for d in range(4):
                    nc.tensor.matmul(
                        ps[: 2 * UCORE, v0 : v0 + vn],
                        gdecT[d][:, : 2 * UCORE],
                        w_bf[:, 4 + d, v0 : v0 + vn],
                        start=(d == 0),
                        stop=(d == 3),
                    )
            nc.vector.tensor_copy(pd_bf[: 2 * UCORE, :], ps[: 2 * UCORE, :])
            rows2 = persist.tile([2, UCORE * V], bf16, tag="rows2")
            nc.scalar.dma_start(rows2[:, :], pd_bf[:, :])

            # P_enc (with bias) per chunk -> pe[0:120, c, :] bf16
            pe = persist.tile([128, NCH, V], bf16, tag="pe")
            for c in range(NCH):
                ps = psum.tile([128, V], f32, tag="bc", bufs=3)
                for v0, vn in vchunks:
                    for d in range(4):
                        nc.tensor.matmul(
                            ps[:PCH, v0 : v0 + vn],
                            gencT[d][:, c * PCH : (c + 1) * PCH],
                            w_bf[:, d, v0 : v0 + vn],
                            start=(d == 0),
                            stop=False,
                        )
                    nc.tensor.matmul(
                        ps[:PCH, v0 : v0 + vn],
                        ones1[0:1, :PCH],
                        bias_sb[:1, v0 : v0 + vn],
                        start=False,
                        stop=True,
                    )
                nc.scalar.copy(pe[:PCH, c, :], ps[:PCH, :])

            # main loop: 13 blocks of 2 u
            for bi, u0 in enumerate(range(0, UCORE, UB)):
                ot = outp.tile([128, UB, NCH, V], bf16, tag="ot", name="ot")
                for j in range(UB):
                    u = u0 + j
                    off = u * V
                    ps = psum.tile([128, V], f32, tag="bc", bufs=3)
                    for c0, cn in vchunks:
                        nc.tensor.matmul(
                            ps[:PCH, c0 : c0 + cn],
                            masks2[0:2, :PCH],
                            rows2[0:2, off + c0 : off + c0 + cn],
                            start=True,
                            stop=True,
                        )
                    bct = bctp.tile([128, V], bf16, tag="bct", name="bct")
                    nc.scalar.copy(bct[:PCH, :], ps[:PCH, :])
                    nc.vector.tensor_add(
                        ot[:PCH, j, :, :],
                        pe[:PCH, :, :],
                        bct[:PCH, :].unsqueeze(1).broadcast_to([PCH, NCH, V]),
                    )
                eng = nc.sync if bi % 2 == 0 else nc.scalar
                if bi == 0 or bi == NBLK - 1:
                    # first block: store per u across both rings so the first
                    # store issues right after the first DVE add; last block:
                    # split across both rings so the drain is half as long
                    nc.sync.dma_start(outD[bi, :, 0, :, :], ot[:PCH, 0, :, :])
                    nc.scalar.dma_start(outD[bi, :, 1, :, :], ot[:PCH, 1, :, :])
                else:
                    eng.dma_start(outD[bi, :, :, :, :], ot[:PCH, :, :, :])

    nc.compile()
    return nc


def kernel(encoder_outputs, decoder_outputs, W_fc, b_fc):
    global LAST_RESULT
    import os

    import ml_dtypes
    from concourse.bass_utils import run_bass_kernel_spmd

    bf = ml_dtypes.bfloat16
    enc = np.asarray(encoder_outputs, dtype=np.float32)
    dec = np.asarray(decoder_outputs, dtype=np.float32)

    # enc per batch-pair -> [120, 5, 512]: partition p = 2*(t%60) + s ...
    # precisely: chunk c, partition p -> batch s = p%2, t = c*60 + p//2
    # (2, 5, 60, D) -[s,c,i]-> transpose to (c, i, s) -> p = 2*i + s
    enc_tiled = np.ascontiguousarray(
        enc.reshape(2, 2, NCH, 60, D)  # [bp, s, c, i, :]
        .transpose(0, 3, 1, 2, 4)  # [bp, i, s, c, :]
        .reshape(2, PCH, NCH, D)
    ).astype(bf)

    # W_fc.T -> [128, 8, 640] with row d = c*128+p
    wT = np.asarray(W_fc, dtype=np.float32).T  # (1024, 640)
    wT_tiled = np.ascontiguousarray(
        wT.reshape(8, 128, V).transpose(1, 0, 2)
    ).astype(bf)

    bias = np.asarray(b_fc, dtype=np.float32)[None, :].astype(bf)

    dec_bf = dec.astype(bf)

    masks2 = np.zeros((2, 128), dtype=bf)
    masks2[0, 0::2] = 1
    masks2[1, 1::2] = 1

    if "nc" not in _cache:
        _cache["nc"] = _build()
    nc = _cache["nc"]

    in_maps = []
    for c in range(NCORES):
        bp, uq = c // 4, c % 4
        u0 = 25 * uq
        in_maps.append(
            {
                "enc": enc_tiled[bp],
                "dec": np.ascontiguousarray(
                    dec_bf[2 * bp : 2 * bp + 2, u0 : u0 + UCORE].reshape(2 * UCORE, D)
                ),
                "wT": wT_tiled,
                "bias": bias,
                "masks2": masks2,
            }
        )

    res = run_bass_kernel_spmd(
        nc,
        in_maps,
        list(range(NCORES)),
        trace=bool(int(os.environ.get("KJ_TRACE", "0"))),
        **RUN_KWARGS,
    )
    LAST_RESULT = res

    out = np.empty((B, T, U, V), dtype=np.float32)
    for c in range(NCORES):
        bp, uq = c // 4, c % 4
        u0 = 25 * uq
        # outD (13,120,2,5,640): [bi,p,j,cc,v] -> s = p%2, t = cc*60 + p//2,
        # u = u0 + 2*bi + j
        arr = res.results[c]["outD"]
        # bf16 -> f32 exact upcast via bit shift (fast)
        f = (arr.view(np.uint16).astype(np.uint32) << 16).view(np.float32)
        # (13, 120, 2, 5, 640) -> (13, 60, 2s, 2j, 5, 640) -> [s, cc, i, bi, j, v]
        cut = np.ascontiguousarray(
            f.reshape(NBLK, 60, 2, UB, NCH, V).transpose(2, 4, 1, 0, 3, 5)
        ).reshape(2, T, UCORE, V)
        lo = 0 if uq == 0 else 1  # quarter q>0: local row 0 is the overlap
        out[2 * bp, :, u0 + lo : u0 + UCORE, :] = cut[0, :, lo:, :]
        out[2 * bp + 1, :, u0 + lo : u0 + UCORE, :] = cut[1, :, lo:, :]
    return out


# revision 5
# speedup vs baseline: 1.1814x; 1.1814x over previous
"""RNN-T JointNet kernel for 8 Trainium2 NeuronCores.

Math: out[b,t,u,:] = gelu_tanh(concat(enc[b,t], dec[b,u])) @ W_fc^T + b_fc
Since gelu is elementwise, gelu(concat(a,b)) = concat(gelu(a), gelu(b)), so
  out[b,t,u,:] = P_enc[b,t,:] + P_dec[b,u,:]
with P_enc = gelu(enc) @ W_fc[:, :512]^T          (small matmul)
     P_dec = gelu(dec) @ W_fc[:, 512:]^T + b_fc   (small matmul; bias folded
                                                   here so it rides the bct)
The dominant cost is streaming the (B,T,U,V) output to HBM. The output is
stored as bf16 (compute is already bf16; rel err ~4e-3) and upcast to f32 on
the host, halving HBM store traffic vs f32 (~20MB/core, ~60us at ~340GB/s).

Sharding: 8 cores = 2 batch-pairs x 4 u-quarters. Core c -> bp = c//4
(batches {0,1} or {2,3}), uq = c%4 with u range [25*uq, 25*uq+26) (26 rows,
1-row overlap between quarters; quarter q>0 contributes local rows 1..25).
Per-core row space: 600 (b,t) rows laid out as 5 chunks of 120 partitions
with the two batches INTERLEAVED across partitions: chunk c, partition p ->
batch 2*bp + p%2, t = c*60 + p//2. This avoids any ragged 44-row t-tail
(no gpsimd elementwise, the old bottleneck), keeps DVE lane utilization at
94%, and keeps SDMA engine load balanced.

The host supplies gelu inputs PRE-TRANSPOSED (encT/decT, feature dim on
partitions) -- gelu commutes with transpose -- so the kernel needs no PE
transposes at all; ACT gelus encT/decT in place and the matmuls read them
directly as lhsT. Per-u pipeline: PE builds P_dec[batch(p), u] + bias into a
[120,640] f32 PSUM tile via a K=2 matmul (even/odd partition masks x the two
batches' P_dec rows from a [2, 26*V] row tile); ACT copies it to a bf16 bct
tile; one DVE tensor_tensor (bf16 2x mode, ~1.8us) adds pe[120,5,640]
(P_enc, bf16) + bct (stride-0 broadcast over the 5 chunks) into the out
tile j-slice. Stores: one 768KB contiguous DMA per u, alternating HWDGE
rings (2 in flight hides the ~2us HBM write-receipt latency), 5 ot bufs.
Block 0 is computed and stored per chunk so the first store issues as soon
as P_enc chunk 0 is done, before chunks 1-4 are computed.
"""

import numpy as np

B, T, U = 4, 300, 101
D = 512
V = 640
UCORE = 26  # u rows per core (4 quarters x 25 + 1 overlap row)
NCORES = 8
UB = 2  # u rows per store block (26 = 13 * 2)
NBLK = UCORE // UB
NCH = 5  # row chunks of 120 partitions (600 = 2 batches x 300 t)
PCH = 120

LAST_RESULT = None  # BassKernelResults of the most recent run (for test.py)
RUN_KWARGS = {}  # extra kwargs test.py may inject (e.g. tmpdir for traces)

_cache = {}


def _build():
    import concourse.mybir as mybir
    from concourse import bacc
    from concourse.tile import TileContext

    f32 = mybir.dt.float32
    bf16 = mybir.dt.bfloat16
    AF = mybir.ActivationFunctionType

    nc = bacc.Bacc()
    # host pre-transposed: encT[p, dch, r] = enc feature d = dch*128+p of row
    # r = c*120 + q  (row -> batch q%2... see module docstring)
    encT_d = nc.dram_tensor("encT", [128, 4, NCH * PCH], bf16, kind="ExternalInput")
    # decT[p, dch, s*26+u] = dec feature d = dch*128+p of (batch s, u0+u)
    decT_d = nc.dram_tensor("decT", [128, 4, 2 * UCORE], bf16, kind="ExternalInput")
    # W_fc.T rows d = dch*128+p; wTd = dec half (cols 512..1023), wTe = enc half
    wTd_d = nc.dram_tensor("wTd", [128, 4, V], bf16, kind="ExternalInput")
    wTe_d = nc.dram_tensor("wTe", [128, 4, V], bf16, kind="ExternalInput")
    bias_d = nc.dram_tensor("bias", [1, V], bf16, kind="ExternalInput")
    # masks2[0, p] = p%2==0, masks2[1, p] = p%2==1 (batch-select for K=2 bcast)
    masks_d = nc.dram_tensor("masks2", [2, 128], bf16, kind="ExternalInput")
    # output laid out exactly like the SBUF tiles so every store is one
    # fully contiguous DRAM write; host un-permutes + upcasts to f32.
    # outD[bi, p, j, c, v] = out[batch p%2, t = c*60 + p//2, u = UB*bi + j, v]
    outD = nc.dram_tensor("outD", [NBLK, PCH, UB, NCH, V], bf16, kind="ExternalOutput")

    vchunks = [(0, 512), (512, V - 512)]

    with TileContext(nc) as tc:
        with (
            tc.tile_pool(name="const", bufs=1) as constp,
            tc.tile_pool(name="persist", bufs=1) as persist,
            tc.tile_pool(name="outp", bufs=5) as outp,
            tc.tile_pool(name="bctp", bufs=4) as bctp,
            tc.tile_pool(name="psum", bufs=1, space="PSUM") as psum,
        ):
            # input loads. sync ring: decT then wTd (they gate the deepest
            # chain P_dec -> rows2 -> bct). scalar ring: encT, consts.
            # wTe on the SWDGE ring overlaps both.
            decT = persist.tile([128, 4, 2 * UCORE], bf16, tag="decT")
            nc.sync.dma_start(decT[:, :, :], decT_d[:, :, :])
            wTd = persist.tile([128, 4, V], bf16, tag="wTd")
            nc.sync.dma_start(wTd[:, :, :], wTd_d[:, :, :])
            encT = persist.tile([128, 4, NCH * PCH], bf16, tag="encT")
            nc.scalar.dma_start(encT[:, :, :], encT_d[:, :, :])
            bias_sb = constp.tile([1, V], bf16)
            nc.scalar.dma_start(bias_sb[:], bias_d[:])
            masks2 = constp.tile([2, 128], bf16)
            nc.scalar.dma_start(masks2[:], masks_d[:])
            wTe = persist.tile([128, 4, V], bf16, tag="wTe")
            nc.gpsimd.dma_start(wTe[:, :, :], wTe_d[:, :, :])

            ones1 = constp.tile([1, 128], bf16)
            nc.gpsimd.memset(ones1[:], 1.0)

            # gelu in the transposed layout (gelu commutes with transpose)
            gdecT = persist.tile([128, 4, 2 * UCORE], bf16, tag="gdecT")
            nc.scalar.activation(gdecT[:, :, :], decT[:, :, :], AF.Gelu_apprx_tanh)
            gencT = persist.tile([128, 4, NCH * PCH], bf16, tag="gencT")
            nc.scalar.activation(gencT[:, :, :], encT[:, :, :], AF.Gelu_apprx_tanh)

            # P_dec + bias -> [52, 640] -> SBUF->SBUF DMA relayout to a
            # [2, 26*V] row tile: rows2[s, u*V + v] = P_dec[batch s, u0+u, v]
            pd_bf = persist.tile([2 * UCORE, V], bf16, tag="pd")
            ps = psum.tile([128, V], f32, tag="bc", bufs=4)
            for v0, vn in vchunks:
                for d in range(4):
                    nc.tensor.matmul(
                        ps[: 2 * UCORE, v0 : v0 + vn],
                        gdecT[:, d, : 2 * UCORE],
                        wTd[:, d, v0 : v0 + vn],
                        start=(d == 0),
                        stop=False,
                    )
                nc.tensor.matmul(
                    ps[: 2 * UCORE, v0 : v0 + vn],
                    ones1[0:1, : 2 * UCORE],
                    bias_sb[:1, v0 : v0 + vn],
                    start=False,
                    stop=True,
                )
            nc.vector.tensor_copy(pd_bf[: 2 * UCORE, :], ps[: 2 * UCORE, :])
            rows2 = persist.tile([2, UCORE * V], bf16, tag="rows2")
            nc.scalar.dma_start(rows2[:, :], pd_bf[:, :])

            pe = persist.tile([128, NCH, V], bf16, tag="pe")

            def enc_chunk(c):
                ps = psum.tile([128, V], f32, tag="bc", bufs=4)
                for v0, vn in vchunks:
                    for d in range(4):
                        nc.tensor.matmul(
                            ps[:PCH, v0 : v0 + vn],
                            gencT[:, d, c * PCH : (c + 1) * PCH],
                            wTe[:, d, v0 : v0 + vn],
                            start=(d == 0),
                            stop=(d == 3),
                        )
                nc.scalar.copy(pe[:PCH, c, :], ps[:PCH, :])

            def bcast(u):
                ps = psum.tile([128, V], f32, tag="bc", bufs=4)
                for c0, cn in vchunks:
                    nc.tensor.matmul(
                        ps[:PCH, c0 : c0 + cn],
                        masks2[0:2, :PCH],
                        rows2[0:2, u * V + c0 : u * V + c0 + cn],
                        start=True,
                        stop=True,
                    )
                bct = bctp.tile([128, V], bf16, tag="bct", name="bct")
                nc.scalar.copy(bct[:PCH, :], ps[:PCH, :])
                return bct

            # P_enc chunk 0 first, then the block-0 broadcasts, then the
            # remaining chunks -- so block 0 (computed per chunk below) can
            # start storing as soon as chunk 0 + rows2 are ready.
            enc_chunk(0)
            bct0 = bcast(0)
            bct1 = bcast(1)
            for c in range(1, NCH):
                enc_chunk(c)

            # block 0: per-chunk adds + stores (stores flow while chunks 1-4
            # are still being computed)
            ot = outp.tile([128, UB, NCH, V], bf16, tag="ot", name="ot")
            for c in range(NCH):
                for j, bct in ((0, bct0), (1, bct1)):
                    nc.vector.tensor_add(
                        ot[:PCH, j, c, :], pe[:PCH, c, :], bct[:PCH, :]
                    )
                eng = nc.sync if c % 2 == 0 else nc.scalar
                eng.dma_start(outD[0, :, :, c, :], ot[:PCH, :, c, :])

            # main loop: blocks 1..12, one DVE add + one 768KB store per u,
            # stores alternating rings so two are always in flight
            for bi in range(1, NBLK):
                ot = outp.tile([128, UB, NCH, V], bf16, tag="ot", name="ot")
                for j in range(UB):
                    u = UB * bi + j
                    bct = bcast(u)
                    nc.vector.tensor_add(
                        ot[:PCH, j, :, :],
                        pe[:PCH, :, :],
                        bct[:PCH, :].unsqueeze(1).broadcast_to([PCH, NCH, V]),
                    )
                    eng = nc.sync if j % 2 == 0 else nc.scalar
                    eng.dma_start(outD[bi, :, j, :, :], ot[:PCH, j, :, :])

    nc.compile()
    return nc


def kernel(encoder_outputs, decoder_outputs, W_fc, b_fc):
    global LAST_RESULT
    import os

    import ml_dtypes
    from concourse.bass_utils import run_bass_kernel_spmd

    bf = ml_dtypes.bfloat16
    enc = np.asarray(encoder_outputs, dtype=np.float32)
    dec = np.asarray(decoder_outputs, dtype=np.float32)

    # row space per batch-pair: r = c*120 + 2*i + s -> batch 2*bp+s, t = c*60+i
    # E[bp, r, :]:
    E = enc.reshape(2, 2, NCH, 60, D).transpose(0, 2, 3, 1, 4).reshape(2, 600, D)
    # encT[bp] = E[bp].T reshaped to [128, 4, 600]
    encT = np.ascontiguousarray(
        E.transpose(0, 2, 1).reshape(2, 4, 128, NCH * PCH).transpose(0, 2, 1, 3)
    ).astype(bf)

    wT = np.asarray(W_fc, dtype=np.float32).T  # (1024, 640)
    wT_tiled = wT.reshape(8, 128, V).transpose(1, 0, 2)  # [128, 8, 640]
    wTe = np.ascontiguousarray(wT_tiled[:, 0:4]).astype(bf)
    wTd = np.ascontiguousarray(wT_tiled[:, 4:8]).astype(bf)

    bias = np.asarray(b_fc, dtype=np.float32)[None, :].astype(bf)

    masks2 = np.zeros((2, 128), dtype=bf)
    masks2[0, 0::2] = 1
    masks2[1, 1::2] = 1

    if "nc" not in _cache:
        _cache["nc"] = _build()
    nc = _cache["nc"]

    in_maps = []
    for c in range(NCORES):
        bp, uq = c // 4, c % 4
        u0 = 25 * uq
        # Dc[s*26+u, :] = dec[2*bp+s, u0+u, :]; decT = Dc.T as [128, 4, 52]
        Dc = dec[2 * bp : 2 * bp + 2, u0 : u0 + UCORE].reshape(2 * UCORE, D)
        decT = np.ascontiguousarray(
            Dc.T.reshape(4, 128, 2 * UCORE).transpose(1, 0, 2)
        ).astype(bf)
        in_maps.append(
            {
                "encT": encT[bp],
                "decT": decT,
                "wTd": wTd,
                "wTe": wTe,
                "bias": bias,
                "masks2": masks2,
            }
        )

    res = run_bass_kernel_spmd(
        nc,
        in_maps,
        list(range(NCORES)),
        trace=bool(int(os.environ.get("KJ_TRACE", "0"))),
        **RUN_KWARGS,
    )
    LAST_RESULT = res

    out = np.empty((B, T, U, V), dtype=np.float32)
    for c in range(NCORES):
        bp, uq = c // 4, c % 4
        u0 = 25 * uq
        # outD (13,120,2,5,640): [bi,p,j,cc,v] -> s = p%2, t = cc*60 + p//2,
        # u = u0 + 2*bi + j
        arr = res.results[c]["outD"]
        # bf16 -> f32 exact upcast via bit shift (fast)
        f = (arr.view(np.uint16).astype(np.uint32) << 16).view(np.float32)
        # (13, 120, 2, 5, 640) -> (13, 60, 2s, 2j, 5, 640) -> [s, cc, i, bi, j, v]
        cut = np.ascontiguousarray(
            f.reshape(NBLK, 60, 2, UB, NCH, V).transpose(2, 4, 1, 0, 3, 5)
        ).reshape(2, T, UCORE, V)
        lo = 0 if uq == 0 else 1  # quarter q>0: local row 0 is the overlap
        out[2 * bp, :, u0 + lo : u0 + UCORE, :] = cut[0, :, lo:, :]
        out[2 * bp + 1, :, u0 + lo : u0 + UCORE, :] = cut[1, :, lo:, :]
    return out


# revision 6
# speedup vs baseline: 1.2499x; 1.0579x over previous
"""RNN-T JointNet kernel for 8 Trainium2 NeuronCores.

Math: out[b,t,u,:] = gelu_tanh(concat(enc[b,t], dec[b,u])) @ W_fc^T + b_fc
Since gelu is elementwise, gelu(concat(a,b)) = concat(gelu(a), gelu(b)), so
  out[b,t,u,:] = P_enc[b,t,:] + P_dec[b,u,:]
with P_enc = gelu(enc) @ W_fc[:, :512]^T          (small matmul)
     P_dec = gelu(dec) @ W_fc[:, 512:]^T + b_fc   (small matmul; bias folded
                                                   here so it rides the bct)
The dominant cost is streaming the (B,T,U,V) output to HBM. The output is
stored as bf16 (compute is already bf16; rel err ~4e-3) and upcast to f32 on
the host, halving HBM store traffic vs f32 (~20MB/core, ~55us at ~360GB/s).

Sharding: 8 cores = 2 batch-pairs x 4 u-quarters. Core c -> bp = c//4
(batches {0,1} or {2,3}), uq = c%4 with u range [25*uq, 25*uq+26) (26 rows,
1-row overlap between quarters; quarter q>0 contributes local rows 1..25).
Per-core row space: 600 (b,t) rows laid out as 5 chunks of 120 partitions
with the two batches INTERLEAVED across partitions: chunk c, partition p ->
batch 2*bp + p%2, t = c*60 + p//2. This avoids any ragged 44-row t-tail
(no gpsimd elementwise), keeps DVE lane utilization at 94%, and keeps SDMA
engine load balanced.

The host supplies gelu inputs PRE-TRANSPOSED (encT/decT, feature dim on
partitions) -- gelu commutes with transpose -- so the kernel needs no PE
transposes; ACT gelus encT/decT in place and the matmuls read them directly
as lhsT. A burst of tiny real matmuls (N=64 off a memset tile) during the
input-load wait warms the PE HAM clock-gate to 2.4GHz before P_dec arrives
(PE transposes would NOT warm it). Per-u pipeline: PE gathers the
per-partition addend P_dec[batch(p), u] + bias straight out of the [52,640]
pd tile into a [120,640] f32 PSUM tile via a K=52 matmul against a
host-loaded one-hot selector slice selU[:, 128u:128u+120] (no SBUF->SBUF
relayout DMA on the critical path); ACT copies it to a bf16 bct tile; one
DVE tensor_tensor (bf16 2x mode, ~1.8us) adds pe[120,5,640] (P_enc, bf16) +
bct (stride-0 broadcast over the 5 chunks) into the out tile j-slice.
Stores: one 768KB fully contiguous DMA per u, alternating HWDGE rings (two
in flight hides the ~2us HBM write-receipt latency), 5 ot bufs. Blocks 0-1
are computed and stored per chunk, interleaved, so stores start as soon as
P_enc chunk 0 is done and flow while chunks 1-4 are still being computed.
"""

import numpy as np

B, T, U = 4, 300, 101
D = 512
V = 640
UCORE = 26  # u rows per core (4 quarters x 25 + 1 overlap row)
NCORES = 8
UB = 2  # u rows per store block (26 = 13 * 2)
NBLK = UCORE // UB
NCH = 5  # row chunks of 120 partitions (600 = 2 batches x 300 t)
PCH = 120
NWARM = 45  # tiny matmuls to hold the PE HAM clock-gate open

LAST_RESULT = None  # BassKernelResults of the most recent run (for test.py)
RUN_KWARGS = {}  # extra kwargs test.py may inject (e.g. tmpdir for traces)

_cache = {}


def _build():
    import concourse.mybir as mybir
    from concourse import bacc
    from concourse.tile import TileContext

    f32 = mybir.dt.float32
    bf16 = mybir.dt.bfloat16
    AF = mybir.ActivationFunctionType

    nc = bacc.Bacc()
    # host pre-transposed: encT[p, dch, r] = enc feature d = dch*128+p of row
    # r = c*120 + q  (row -> batch q%2... see module docstring)
    encT_d = nc.dram_tensor("encT", [128, 4, NCH * PCH], bf16, kind="ExternalInput")
    # decT[p, dch, s*26+u] = dec feature d = dch*128+p of (batch s, u0+u)
    decT_d = nc.dram_tensor("decT", [128, 4, 2 * UCORE], bf16, kind="ExternalInput")
    # W_fc.T rows d = dch*128+p; wTd = dec half (cols 512..1023), wTe = enc half
    wTd_d = nc.dram_tensor("wTd", [128, 4, V], bf16, kind="ExternalInput")
    wTe_d = nc.dram_tensor("wTe", [128, 4, V], bf16, kind="ExternalInput")
    bias_d = nc.dram_tensor("bias", [1, V], bf16, kind="ExternalInput")
    # selU[r, 128u + p] = 1 iff r == (p%2)*26 + u: K=52 gather-broadcast masks
    selU_d = nc.dram_tensor("selU", [2 * UCORE, UCORE * 128], bf16, kind="ExternalInput")
    # output laid out exactly like the SBUF tiles so every store is one
    # fully contiguous DRAM write; host un-permutes + upcasts to f32.
    # outD[bi, p, j, c, v] = out[batch p%2, t = c*60 + p//2, u = UB*bi + j, v]
    outD = nc.dram_tensor("outD", [NBLK, PCH, UB, NCH, V], bf16, kind="ExternalOutput")

    vchunks = [(0, 512), (512, V - 512)]

    with TileContext(nc) as tc:
        with (
            tc.tile_pool(name="const", bufs=1) as constp,
            tc.tile_pool(name="persist", bufs=1) as persist,
            tc.tile_pool(name="outp", bufs=5) as outp,
            tc.tile_pool(name="bctp", bufs=4) as bctp,
            tc.tile_pool(name="psum", bufs=1, space="PSUM") as psum,
        ):
            # input loads. sync ring: decT then wTd (they gate the deepest
            # chain gelu -> P_dec -> bct). scalar ring: encT, consts.
            # wTe on the SWDGE ring overlaps both.
            decT = persist.tile([128, 4, 2 * UCORE], bf16, tag="decT")
            nc.sync.dma_start(decT[:, :, :], decT_d[:, :, :])
            wTd = persist.tile([128, 4, V], bf16, tag="wTd")
            nc.sync.dma_start(wTd[:, :, :], wTd_d[:, :, :])
            encT = persist.tile([128, 4, NCH * PCH], bf16, tag="encT")
            nc.scalar.dma_start(encT[:, :, :], encT_d[:, :, :])
            bias_sb = constp.tile([1, V], bf16)
            nc.scalar.dma_start(bias_sb[:], bias_d[:])
            selU = constp.tile([2 * UCORE, UCORE * 128], bf16)
            nc.scalar.dma_start(selU[:, :], selU_d[:, :])
            ones1 = constp.tile([1, 128], bf16)
            nc.gpsimd.memset(ones1[:], 1.0)
            wTe = persist.tile([128, 4, V], bf16, tag="wTe")
            nc.gpsimd.dma_start(wTe[:, :, :], wTe_d[:, :, :])

            # warm the PE HAM clock-gate with tiny REAL matmuls while the
            # input DMAs land (the HAM watches matmul busy time; ~3.4us of
            # sustained activity lifts the PE from 1.2 to 2.4 GHz)
            for _ in range(NWARM):
                wm = psum.tile([1, 64], f32, tag="wm", bufs=2)
                nc.tensor.matmul(wm[0:1, :], ones1[0:1, 0:1], ones1[0:1, 0:64], start=True, stop=True)

            # gelu in the transposed layout (gelu commutes with transpose)
            gdecT = persist.tile([128, 4, 2 * UCORE], bf16, tag="gdecT")
            nc.scalar.activation(gdecT[:, :, :], decT[:, :, :], AF.Gelu_apprx_tanh)
            gencT = persist.tile([128, 4, NCH * PCH], bf16, tag="gencT")
            nc.scalar.activation(gencT[:, :, :], encT[:, :, :], AF.Gelu_apprx_tanh)

            # P_dec + bias -> pd [52, 640] bf16 (kept partition-major; the
            # per-u broadcast gathers rows straight from it via selU)
            pd_bf = persist.tile([2 * UCORE, V], bf16, tag="pd")
            ps = psum.tile([128, V], f32, tag="bc", bufs=3)
            for v0, vn in vchunks:
                for d in range(4):
                    nc.tensor.matmul(
                        ps[: 2 * UCORE, v0 : v0 + vn],
                        gdecT[:, d, : 2 * UCORE],
                        wTd[:, d, v0 : v0 + vn],
                        start=(d == 0),
                        stop=False,
                    )
                nc.tensor.matmul(
                    ps[: 2 * UCORE, v0 : v0 + vn],
                    ones1[0:1, : 2 * UCORE],
                    bias_sb[:1, v0 : v0 + vn],
                    start=False,
                    stop=True,
                )
            nc.vector.tensor_copy(pd_bf[: 2 * UCORE, :], ps[: 2 * UCORE, :])

            pe = persist.tile([128, NCH, V], bf16, tag="pe")

            def enc_chunk(c):
                ps = psum.tile([128, V], f32, tag="bc", bufs=3)
                for v0, vn in vchunks:
                    for d in range(4):
                        nc.tensor.matmul(
                            ps[:PCH, v0 : v0 + vn],
                            gencT[:, d, c * PCH : (c + 1) * PCH],
                            wTe[:, d, v0 : v0 + vn],
                            start=(d == 0),
                            stop=(d == 3),
                        )
                nc.scalar.copy(pe[:PCH, c, :], ps[:PCH, :])

            def bcast(u):
                ps = psum.tile([128, V], f32, tag="bc", bufs=3)
                for c0, cn in vchunks:
                    nc.tensor.matmul(
                        ps[:PCH, c0 : c0 + cn],
                        selU[:, u * 128 : u * 128 + PCH],
                        pd_bf[:, c0 : c0 + cn],
                        start=True,
                        stop=True,
                    )
                bct = bctp.tile([128, V], bf16, tag="bct", name="bct")
                nc.scalar.copy(bct[:PCH, :], ps[:PCH, :])
                return bct

            # P_enc chunk 0 first, then the broadcasts for blocks 0-1, then
            # the remaining chunks -- blocks 0-1 are computed per chunk below
            # so stores start as soon as chunk 0 + pd are ready.
            enc_chunk(0)
            bcts = [bcast(u) for u in range(4)]
            ots = [
                outp.tile([128, UB, NCH, V], bf16, tag="ot", name=f"ot0{b}")
                for b in range(2)
            ]
            for c in range(NCH):
                if c > 0:
                    enc_chunk(c)
                for b in range(2):
                    for j in range(UB):
                        nc.vector.tensor_add(
                            ots[b][:PCH, j, c, :],
                            pe[:PCH, c, :],
                            bcts[UB * b + j][:PCH, :],
                        )
                    eng = nc.sync if c % 2 == b else nc.scalar
                    eng.dma_start(outD[b, :, :, c, :], ots[b][:PCH, :, c, :])

            # main loop: blocks 2..12, one DVE add + one 768KB store per u,
            # stores alternating rings so two are always in flight
            for bi in range(2, NBLK):
                ot = outp.tile([128, UB, NCH, V], bf16, tag="ot", name="ot")
                for j in range(UB):
                    u = UB * bi + j
                    bct = bcast(u)
                    nc.vector.tensor_add(
                        ot[:PCH, j, :, :],
                        pe[:PCH, :, :],
                        bct[:PCH, :].unsqueeze(1).broadcast_to([PCH, NCH, V]),
                    )
                    eng = nc.sync if j % 2 == 0 else nc.scalar
                    eng.dma_start(outD[bi, :, j, :, :], ot[:PCH, j, :, :])

    nc.compile()
    return nc


def kernel(encoder_outputs, decoder_outputs, W_fc, b_fc):
    global LAST_RESULT
    import os

    import ml_dtypes
    from concourse.bass_utils import run_bass_kernel_spmd

    bf = ml_dtypes.bfloat16
    enc = np.asarray(encoder_outputs, dtype=np.float32)
    dec = np.asarray(decoder_outputs, dtype=np.float32)

    # row space per batch-pair: r = c*120 + 2*i + s -> batch 2*bp+s, t = c*60+i
    E = enc.reshape(2, 2, NCH, 60, D).transpose(0, 2, 3, 1, 4).reshape(2, 600, D)
    # encT[bp] = E[bp].T reshaped to [128, 4, 600]
    encT = np.ascontiguousarray(
        E.transpose(0, 2, 1).reshape(2, 4, 128, NCH * PCH).transpose(0, 2, 1, 3)
    ).astype(bf)

    wT = np.asarray(W_fc, dtype=np.float32).T  # (1024, 640)
    wT_tiled = wT.reshape(8, 128, V).transpose(1, 0, 2)  # [128, 8, 640]
    wTe = np.ascontiguousarray(wT_tiled[:, 0:4]).astype(bf)
    wTd = np.ascontiguousarray(wT_tiled[:, 4:8]).astype(bf)

    bias = np.asarray(b_fc, dtype=np.float32)[None, :].astype(bf)

    # selU[r, 128u + p] = 1 iff r == (p%2)*26 + u
    selU = np.zeros((2 * UCORE, UCORE * 128), dtype=bf)
    for u in range(UCORE):
        selU[u, u * 128 + 0 : (u + 1) * 128 : 2] = 1
        selU[UCORE + u, u * 128 + 1 : (u + 1) * 128 : 2] = 1

    if "nc" not in _cache:
        _cache["nc"] = _build()
    nc = _cache["nc"]

    in_maps = []
    for c in range(NCORES):
        bp, uq = c // 4, c % 4
        u0 = 25 * uq
        # Dc[s*26+u, :] = dec[2*bp+s, u0+u, :]; decT = Dc.T as [128, 4, 52]
        Dc = dec[2 * bp : 2 * bp + 2, u0 : u0 + UCORE].reshape(2 * UCORE, D)
        decT = np.ascontiguousarray(
            Dc.T.reshape(4, 128, 2 * UCORE).transpose(1, 0, 2)
        ).astype(bf)
        in_maps.append(
            {
                "encT": encT[bp],
                "decT": decT,
                "wTd": wTd,
                "wTe": wTe,
                "bias": bias,
                "selU": selU,
            }
        )

    res = run_bass_kernel_spmd(
        nc,
        in_maps,
        list(range(NCORES)),
        trace=bool(int(os.environ.get("KJ_TRACE", "0"))),
        **RUN_KWARGS,
    )
    LAST_RESULT = res

    out = np.empty((B, T, U, V), dtype=np.float32)
    for c in range(NCORES):
        bp, uq = c // 4, c % 4
        u0 = 25 * uq
        # outD (13,120,2,5,640): [bi,p,j,cc,v] -> s = p%2, t = cc*60 + p//2,
        # u = u0 + 2*bi + j
        arr = res.results[c]["outD"]
        # bf16 -> f32 exact upcast via bit shift (fast)
        f = (arr.view(np.uint16).astype(np.uint32) << 16).view(np.float32)
        cut = np.ascontiguousarray(
            f.reshape(NBLK, 60, 2, UB, NCH, V).transpose(2, 4, 1, 0, 3, 5)
        ).reshape(2, T, UCORE, V)
        lo = 0 if uq == 0 else 1  # quarter q>0: local row 0 is the overlap
        out[2 * bp, :, u0 + lo : u0 + UCORE, :] = cut[0, :, lo:, :]
        out[2 * bp + 1, :, u0 + lo : u0 + UCORE, :] = cut[1, :, lo:, :]
    return out


# revision 10
# speedup vs baseline: 1.2608x; 1.0087x over previous
"""RNN-T JointNet kernel for 8 Trainium2 NeuronCores.

Math: out[b,t,u,:] = gelu_tanh(concat(enc[b,t], dec[b,u])) @ W_fc^T + b_fc
Since gelu is elementwise, gelu(concat(a,b)) = concat(gelu(a), gelu(b)), so
  out[b,t,u,:] = P_enc[b,t,:] + P_dec[b,u,:]
with P_enc = gelu(enc) @ W_fc[:, :512]^T          (small matmul)
     P_dec = gelu(dec) @ W_fc[:, 512:]^T + b_fc   (small matmul; bias folded
                                                   here so it rides the bct)
The dominant cost is streaming the (B,T,U,V) output to HBM. The output is
stored as bf16 (compute is already bf16; rel err ~4e-3) and upcast to f32 on
the host, halving HBM store traffic vs f32 (~20MB/core, ~55us at ~360GB/s).

Sharding: 8 cores = 2 batch-pairs x 4 u-quarters. Core c -> bp = c//4
(batches {0,1} or {2,3}), uq = c%4 with u range [25*uq, 25*uq+26) (26 rows,
1-row overlap between quarters; quarter q>0 contributes local rows 1..25).
Per-core row space: 600 (b,t) rows laid out as 5 chunks of 120 partitions
with the two batches INTERLEAVED across partitions: chunk c, partition p ->
batch 2*bp + p%2, t = c*60 + p//2. This avoids any ragged 44-row t-tail
(no gpsimd elementwise), keeps DVE lane utilization at 94%, and keeps SDMA
engine load balanced.

The host supplies gelu inputs PRE-TRANSPOSED (encT/decT, feature dim on
partitions) -- gelu commutes with transpose -- so the kernel needs no PE
transposes; ACT gelus encT/decT in place and the matmuls read them directly
as lhsT. A burst of tiny real matmuls (N=64 off a memset tile) during the
input-load wait warms the PE HAM clock-gate to 2.4GHz before P_dec arrives
(PE transposes would NOT warm it). Per-u pipeline: PE gathers the
per-partition addend P_dec[batch(p), u] + bias straight out of the [52,640]
pd tile into a [120,640] f32 PSUM tile via a K=52 matmul against a
host-loaded one-hot selector slice selU[:, 128u:128u+120] (no SBUF->SBUF
relayout DMA on the critical path); ACT copies it to a bf16 bct tile; one
DVE tensor_tensor (bf16 2x mode, ~1.8us) adds pe[120,5,640] (P_enc, bf16) +
bct (stride-0 broadcast over the 5 chunks) into the out tile j-slice.
Stores: one 768KB fully contiguous DMA per u, alternating HWDGE rings (two
in flight hides the ~2us HBM write-receipt latency), 5 ot bufs. Blocks 0-1
are computed and stored per chunk, interleaved, so stores start as soon as
P_enc chunk 0 is done and flow while chunks 1-4 are still being computed.
"""

import numpy as np

B, T, U = 4, 300, 101
D = 512
V = 640
UCORE = 26  # u rows per core (4 quarters x 25 + 1 overlap row)
NCORES = 8
UB = 2  # u rows per store block (26 = 13 * 2)
NBLK = UCORE // UB
NCH = 5  # row chunks of 120 partitions (600 = 2 batches x 300 t)
PCH = 120
NWARM = 30  # tiny matmuls to hold the PE HAM clock-gate open

LAST_RESULT = None  # BassKernelResults of the most recent run (for test.py)
RUN_KWARGS = {}  # extra kwargs test.py may inject (e.g. tmpdir for traces)

_cache = {}


def _build():
    import concourse.mybir as mybir
    from concourse import bacc
    from concourse.tile import TileContext

    f32 = mybir.dt.float32
    bf16 = mybir.dt.bfloat16
    AF = mybir.ActivationFunctionType

    nc = bacc.Bacc()
    # host pre-transposed: encT[p, dch, r] = enc feature d = dch*128+p of row
    # r = c*120 + q  (row -> batch q%2... see module docstring)
    encT_d = nc.dram_tensor("encT", [128, 4, NCH * PCH], bf16, kind="ExternalInput")
    # decT[p, dch, s*26+u] = dec feature d = dch*128+p of (batch s, u0+u)
    decT_d = nc.dram_tensor("decT", [128, 4, 2 * UCORE], bf16, kind="ExternalInput")
    # W_fc.T rows d = dch*128+p; wTd = dec half (cols 512..1023), wTe = enc half
    wTd_d = nc.dram_tensor("wTd", [128, 4, V], bf16, kind="ExternalInput")
    wTe_d = nc.dram_tensor("wTe", [128, 4, V], bf16, kind="ExternalInput")
    bias_d = nc.dram_tensor("bias", [1, V], bf16, kind="ExternalInput")
    # selU[r, 128u + p] = 1 iff r == (p%2)*26 + u: K=52 gather-broadcast masks
    selU_d = nc.dram_tensor("selU", [2 * UCORE, UCORE * 128], bf16, kind="ExternalInput")
    # output laid out exactly like the SBUF tiles so every store is one
    # fully contiguous DRAM write; host un-permutes + upcasts to f32.
    # outD[bi, p, j, c, v] = out[batch p%2, t = c*60 + p//2, u = UB*bi + j, v]
    outD = nc.dram_tensor("outD", [NBLK, PCH, UB, NCH, V], bf16, kind="ExternalOutput")

    vchunks = [(0, 512), (512, V - 512)]

    with TileContext(nc) as tc:
        with (
            tc.tile_pool(name="const", bufs=1) as constp,
            tc.tile_pool(name="persist", bufs=1) as persist,
            tc.tile_pool(name="outp", bufs=5) as outp,
            tc.tile_pool(name="bctp", bufs=4) as bctp,
            tc.tile_pool(name="psum", bufs=1, space="PSUM") as psum,
        ):
            # input loads. sync ring: decT, then wTd per d-chunk (P_dec runs
            # d-major and starts on chunk 0 before the rest land), then the
            # small consts. scalar ring carries ONLY the encT issue so the
            # ACT sequencer gets to its gelu table loads immediately. wTe per
            # d-chunk on the SWDGE ring overlaps both.
            decT = persist.tile([128, 4, 2 * UCORE], bf16, tag="decT")
            nc.sync.dma_start(decT[:, :, :], decT_d[:, :, :])
            wTd = persist.tile([128, 4, V], bf16, tag="wTd")
            for dch in range(4):
                nc.sync.dma_start(wTd[:, dch, :], wTd_d[:, dch, :])
            encT = persist.tile([128, 4, NCH * PCH], bf16, tag="encT")
            nc.scalar.dma_start(encT[:, :, :], encT_d[:, :, :])
            bias_sb = constp.tile([1, V], bf16)
            nc.sync.dma_start(bias_sb[:], bias_d[:])
            selU = constp.tile([2 * UCORE, UCORE * 128], bf16)
            nc.sync.dma_start(selU[:, :], selU_d[:, :])
            ones1 = constp.tile([1, 128], bf16)
            nc.gpsimd.memset(ones1[:], 1.0)
            wTe = persist.tile([128, 4, V], bf16, tag="wTe")
            for dch in range(4):
                nc.gpsimd.dma_start(wTe[:, dch, :], wTe_d[:, dch, :])

            # warm the PE HAM clock-gate with tiny REAL matmuls while the
            # input DMAs land (the HAM watches matmul busy time; ~3.4us of
            # sustained activity lifts the PE from 1.2 to 2.4 GHz)
            for _ in range(NWARM):
                wm = psum.tile([1, 64], f32, tag="wm", bufs=2)
                nc.tensor.matmul(wm[0:1, :], ones1[0:1, 0:1], ones1[0:1, 0:64], start=True, stop=True)

            # gelu in the transposed layout (gelu commutes with transpose)
            gdecT = persist.tile([128, 4, 2 * UCORE], bf16, tag="gdecT")
            nc.scalar.activation(gdecT[:, :, :], decT[:, :, :], AF.Gelu_apprx_tanh)
            gencT = persist.tile([128, 4, NCH * PCH], bf16, tag="gencT")
            nc.scalar.activation(gencT[:, :, :], encT[:, :, :], AF.Gelu_apprx_tanh)

            # P_dec + bias -> pd [52, 640] bf16 (kept partition-major; the
            # per-u broadcast gathers rows straight from it via selU).
            # d-major so matmuls start as soon as wTd chunk 0 lands.
            pd_bf = persist.tile([2 * UCORE, V], bf16, tag="pd")
            ps = psum.tile([128, V], f32, tag="bc", bufs=3)
            for d in range(4):
                for v0, vn in vchunks:
                    nc.tensor.matmul(
                        ps[: 2 * UCORE, v0 : v0 + vn],
                        gdecT[:, d, : 2 * UCORE],
                        wTd[:, d, v0 : v0 + vn],
                        start=(d == 0),
                        stop=False,
                    )
            for v0, vn in vchunks:
                nc.tensor.matmul(
                    ps[: 2 * UCORE, v0 : v0 + vn],
                    ones1[0:1, : 2 * UCORE],
                    bias_sb[:1, v0 : v0 + vn],
                    start=False,
                    stop=True,
                )
            nc.vector.tensor_copy(pd_bf[: 2 * UCORE, :], ps[: 2 * UCORE, :])

            pe = persist.tile([128, NCH, V], bf16, tag="pe")

            def enc_chunk(c):
                ps = psum.tile([128, V], f32, tag="bc", bufs=3)
                for d in range(4):
                    for v0, vn in vchunks:
                        nc.tensor.matmul(
                            ps[:PCH, v0 : v0 + vn],
                            gencT[:, d, c * PCH : (c + 1) * PCH],
                            wTe[:, d, v0 : v0 + vn],
                            start=(d == 0),
                            stop=(d == 3),
                        )
                nc.scalar.copy(pe[:PCH, c, :], ps[:PCH, :])

            def bcast(u):
                ps = psum.tile([128, V], f32, tag="bc", bufs=3)
                for c0, cn in vchunks:
                    nc.tensor.matmul(
                        ps[:PCH, c0 : c0 + cn],
                        selU[:, u * 128 : u * 128 + PCH],
                        pd_bf[:, c0 : c0 + cn],
                        start=True,
                        stop=True,
                    )
                bct = bctp.tile([128, V], bf16, tag="bct", name="bct")
                nc.scalar.copy(bct[:PCH, :], ps[:PCH, :])
                return bct

            # P_enc chunk 0 first, then the broadcasts for blocks 0-1, then
            # the remaining chunks -- blocks 0-1 are computed per chunk below
            # so stores start as soon as chunk 0 + pd are ready.
            enc_chunk(0)
            bcts = [bcast(u) for u in range(4)]
            ots = [
                outp.tile([128, UB, NCH, V], bf16, tag="ot", name=f"ot0{b}")
                for b in range(2)
            ]
            for c in range(NCH):
                if c > 0:
                    enc_chunk(c)
                for b in range(2):
                    for j in range(UB):
                        nc.vector.tensor_add(
                            ots[b][:PCH, j, c, :],
                            pe[:PCH, c, :],
                            bcts[UB * b + j][:PCH, :],
                        )
                    eng = nc.sync if c % 2 == b else nc.scalar
                    eng.dma_start(outD[b, :, :, c, :], ots[b][:PCH, :, c, :])

            # main loop: blocks 2..12, one DVE add + one 768KB store per u,
            # stores alternating rings so two are always in flight
            for bi in range(2, NBLK):
                ot = outp.tile([128, UB, NCH, V], bf16, tag="ot", name="ot")
                for j in range(UB):
                    u = UB * bi + j
                    bct = bcast(u)
                    nc.vector.tensor_add(
                        ot[:PCH, j, :, :],
                        pe[:PCH, :, :],
                        bct[:PCH, :].unsqueeze(1).broadcast_to([PCH, NCH, V]),
                    )
                    eng = nc.sync if j % 2 == 0 else nc.scalar
                    eng.dma_start(outD[bi, :, j, :, :], ot[:PCH, j, :, :])

    nc.compile()
    return nc


def kernel(encoder_outputs, decoder_outputs, W_fc, b_fc):
    global LAST_RESULT
    import os

    import ml_dtypes
    from concourse.bass_utils import run_bass_kernel_spmd

    bf = ml_dtypes.bfloat16
    enc = np.asarray(encoder_outputs, dtype=np.float32)
    dec = np.asarray(decoder_outputs, dtype=np.float32)

    # row space per batch-pair: r = c*120 + 2*i + s -> batch 2*bp+s, t = c*60+i
    E = enc.reshape(2, 2, NCH, 60, D).transpose(0, 2, 3, 1, 4).reshape(2, 600, D)
    # encT[bp] = E[bp].T reshaped to [128, 4, 600]
    encT = np.ascontiguousarray(
        E.transpose(0, 2, 1).reshape(2, 4, 128, NCH * PCH).transpose(0, 2, 1, 3)
    ).astype(bf)

    wT = np.asarray(W_fc, dtype=np.float32).T  # (1024, 640)
    wT_tiled = wT.reshape(8, 128, V).transpose(1, 0, 2)  # [128, 8, 640]
    wTe = np.ascontiguousarray(wT_tiled[:, 0:4]).astype(bf)
    wTd = np.ascontiguousarray(wT_tiled[:, 4:8]).astype(bf)

    bias = np.asarray(b_fc, dtype=np.float32)[None, :].astype(bf)

    # selU[r, 128u + p] = 1 iff r == (p%2)*26 + u
    selU = np.zeros((2 * UCORE, UCORE * 128), dtype=bf)
    for u in range(UCORE):
        selU[u, u * 128 + 0 : (u + 1) * 128 : 2] = 1
        selU[UCORE + u, u * 128 + 1 : (u + 1) * 128 : 2] = 1

    if "nc" not in _cache:
        _cache["nc"] = _build()
    nc = _cache["nc"]

    in_maps = []
    for c in range(NCORES):
        bp, uq = c // 4, c % 4
        u0 = 25 * uq
        # Dc[s*26+u, :] = dec[2*bp+s, u0+u, :]; decT = Dc.T as [128, 4, 52]
        Dc = dec[2 * bp : 2 * bp + 2, u0 : u0 + UCORE].reshape(2 * UCORE, D)
        decT = np.ascontiguousarray(
            Dc.T.reshape(4, 128, 2 * UCORE).transpose(1, 0, 2)
        ).astype(bf)
        in_maps.append(
            {
                "encT": encT[bp],
                "decT": decT,
                "wTd": wTd,
                "wTe": wTe,
                "bias": bias,
                "selU": selU,
            }
        )

    res = run_bass_kernel_spmd(
        nc,
        in_maps,
        list(range(NCORES)),
        trace=bool(int(os.environ.get("KJ_TRACE", "0"))),
        **RUN_KWARGS,
    )
    LAST_RESULT = res

    out = np.empty((B, T, U, V), dtype=np.float32)
    for c in range(NCORES):
        bp, uq = c // 4, c % 4
        u0 = 25 * uq
        # outD (13,120,2,5,640): [bi,p,j,cc,v] -> s = p%2, t = cc*60 + p//2,
        # u = u0 + 2*bi + j
        arr = res.results[c]["outD"]
        # bf16 -> f32 exact upcast via bit shift (fast)
        f = (arr.view(np.uint16).astype(np.uint32) << 16).view(np.float32)
        cut = np.ascontiguousarray(
            f.reshape(NBLK, 60, 2, UB, NCH, V).transpose(2, 4, 1, 0, 3, 5)
        ).reshape(2, T, UCORE, V)
        lo = 0 if uq == 0 else 1  # quarter q>0: local row 0 is the overlap
        out[2 * bp, :, u0 + lo : u0 + UCORE, :] = cut[0, :, lo:, :]
        out[2 * bp + 1, :, u0 + lo : u0 + UCORE, :] = cut[1, :, lo:, :]
    return out
